# revision 27
# baseline (speedup 1.0000x reference)
"""Bass/Tile TRN2 kernel for nn_Custom_Dropout (zero out NUM_BOXES rectangles
per (batch, channel) image).

Contract: kernel(**inputs) takes FULL inputs (x [32,3,512,512] f32,
width_positions/height_positions [32,3,8,2] i32) and returns the FULL
[32,3,512,512] f32 output. Internally shards batch across 8 NeuronCores
(pure data parallel, 4 batches -> 12 images of 512x512 per core).

The kernel is DMA-engine-bound (16 SDMA engines shared by all queues,
~23-28 B/ns each), so x travels on the wire as bf16 (host casts f32 -> bf16
when sharding, upcasts the result back to f32; bf16 rounding is ~2^-9
relative, well inside the 2e-2 gate) and images are packed in PAIRS so
every DMA descriptor is one 8 KiB contiguous DRAM block per partition:

  pair tile [128, 8, 512]: partitions 0-63 hold image A (w = 8p + r),
  partitions 64-127 hold image B (w = 8(p-64) + r).

The per-row box masks are a tiny O((W+H)*boxes) re-encoding of the box
index tensors, so the host builds them directly (fp8, 0/1 exact) while
sharding; the O(W*H) coverage-count and select run on device:

  maskw[32q+k, j] (j in [0,1024)): box k of image A for k<8 (j<512 region),
    box k-8 of image B for k in [8,16) (j-512 region). The lhsT slice
    maskw[:, r::8] then covers out rows m<64 -> A, m>=64 -> B with the
    other image's half automatically 0.
  maskh[32q+k, h]: plain per-box h masks (A at k<8, B at k>=8).
  cnt[m, h] = sum_k maskw[k, 8m+r] * maskh[k, h]   (K=16 fp8 matmul -> f32
    PSUM, one matmul per r, M=128 covers both images of the pair)
  then per [128, 4, 512] half-pair, one of three select paths (balancing
  engine occupancy; HW-measured: ACT relu 2.0us, DVE tensor_tensor-bf16
  1.2us (2x mode), DVE STT-from-PSUM 2.3us (1x; the scalar port blocks
  DVE fast modes for STT), Pool TT 4.1us):
    'A': ACT keep = Relu(1 - cnt)  (PSUM -> bf16 SBUF), DVE out = keep * x
    'P': ACT keep as above,        Pool (gpsimd) out = keep * x
    'D': DVE out = (cnt <= 0) * x  (one scalar_tensor_tensor at 1x)

Out-DMA issues live on the GpSimd SWDGE stream (it has slack, so an issue
waiting for a slow half never blocks the convert chain, unlike on ACT),
deferred one pair so issue order tracks output readiness.
"""

import ml_dtypes
import numpy as np

import concourse.bass as bass
import concourse.bacc as bacc
import concourse.mybir as mybir
import concourse.tile as tile
from concourse.bass_utils import run_bass_kernel_spmd

N_CORES = 8
B, C, W, H = 32, 3, 512, 512
BL = B // N_CORES        # batches per core
NI = BL * C              # images per core
NP = NI // 2             # image pairs per core
NB = 8                   # boxes per image
NT = (NP + 3) // 4       # mask tile-sets (4 pairs each)
R = 8                    # w rows per partition (64 partitions per image)

_DT = mybir.dt
_ALU = mybir.AluOpType
_F8 = mybir.dt.np(mybir.dt.float8e4)
# select path per (pair, half): 'A' = ACT convert + DVE TT mult,
# 'P' = ACT convert + Pool TT mult, 'D' = direct DVE STT from PSUM.
PATHS = {
    (0, 0): "A", (0, 1): "A",
    (1, 0): "A", (1, 1): "D",
    (2, 0): "A", (2, 1): "P",
    (3, 0): "D", (3, 1): "D",
    (4, 0): "A", (4, 1): "P",
    (5, 0): "A", (5, 1): "D",
}


def build_bass():
    nc = bacc.Bacc(
        "TRN2",
        debug=False,
        target_bir_lowering=False,
        num_devices=N_CORES,
    )
    x_in = nc.dram_tensor("x", [BL, C, W, H], _DT.bfloat16, kind="ExternalInput")
    mw_in = nc.dram_tensor("mw", [128, NT, 2 * W], _DT.float8e4, kind="ExternalInput")
    mh_in = nc.dram_tensor("mh", [128, NT, H], _DT.float8e4, kind="ExternalInput")
    out = nc.dram_tensor("out", [BL, C, W, H], _DT.bfloat16, kind="ExternalOutput")

    # w = p*R + r: 8 KiB contiguous per partition per image
    xflat = x_in.rearrange("b c (p r) h -> (b c) p r h", r=R)
    oflat = out.rearrange("b c (p r) h -> (b c) p r h", r=R)

    def pair_ap(flat, p):
        return flat[2 * p : 2 * p + 2].rearrange("two p r h -> (two p) r h")

    with tile.TileContext(nc) as tc:
        with (
            tc.tile_pool(name="const", bufs=1) as constp,
            tc.tile_pool(name="xio", bufs=1) as xp,
            tc.tile_pool(name="oio", bufs=4) as op,
            tc.tile_pool(name="keep", bufs=2) as kp,
            tc.tile_pool(name="psum", bufs=1, space="PSUM") as pp,
        ):
            # masks go FIRST on the Sync ring (tiny, gate the matmuls)
            mw_sb = constp.tile([128, NT, 2 * W], _DT.float8e4)
            mh_sb = constp.tile([128, NT, H], _DT.float8e4)
            nc.sync.dma_start(mw_sb[:], mw_in[:])
            nc.sync.dma_start(mh_sb[:], mh_in[:])
            # all input pair DMAs (1 MiB each) follow on the Sync ring;
            # all 6 pair tiles stay resident (48 KiB/partition) so the
            # input stream never stalls on buffer reuse.
            x_tiles = []
            for p in range(NP):
                x_t = xp.tile([128, R, H], _DT.bfloat16, tag=f"x{p}")
                nc.sync.dma_start(x_t[:], pair_ap(xflat, p))
                x_tiles.append(x_t)
            # dummy read on the GpSimd ring: absorbs its queue cold-start
            # during the preamble so the first real out-DMA flows at once.
            warm_gb = constp.tile([128, 1, 4], _DT.float8e4)
            nc.gpsimd.dma_start(warm_gb[:], mh_in[:, 0:1, 0:4])

            pending_out = []
            for p in range(NP):
                T, q = divmod(p, 4)
                x_t = x_tiles[p]
                o_t = op.tile([128, R, H], _DT.bfloat16, tag="o")
                for half in range(2):
                    cnt = pp.tile([128, 4, H], _DT.float32, tag=f"c{half}", bufs=1)
                    for rl in range(4):
                        r = 4 * half + rl
                        nc.tensor.matmul(
                            cnt[:, rl, :],
                            mw_sb[32 * q : 32 * q + 2 * NB, T, r::R],
                            mh_sb[32 * q : 32 * q + 2 * NB, T, :],
                            tile_position=(32 * q, 0),
                        )
                    xs = x_t[:, 4 * half : 4 * half + 4, :]
                    os = o_t[:, 4 * half : 4 * half + 4, :]
                    path = PATHS[(p, half)]
                    if path == "D":
                        nc.vector.scalar_tensor_tensor(
                            os, cnt[:], 0.0, xs, _ALU.is_le, _ALU.mult,
                        )
                    else:
                        keep = kp.tile([128, 4, H], _DT.bfloat16, tag=f"k{half}")
                        nc.scalar.activation(
                            keep[:], cnt[:], mybir.ActivationFunctionType.Relu,
                            bias=1.0, scale=-1.0,
                        )
                        eng = nc.vector if path == "A" else nc.gpsimd
                        eng.tensor_tensor(os, keep[:], xs, _ALU.mult)
                # Out-DMA issues on the GpSimd (SWDGE) stream, deferred one
                # pair so the issue order tracks output readiness.
                pending_out.append([pair_ap(oflat, p), o_t[:], 1])
                while pending_out and pending_out[0][2] <= 0:
                    dst, src, _ = pending_out.pop(0)
                    nc.gpsimd.dma_start(dst, src)
                for ent in pending_out:
                    ent[2] -= 1
            for dst, src, _ in pending_out:
                nc.gpsimd.dma_start(dst, src)

    nc.compile()
    return nc


_CACHED_NC = None


def _get_nc():
    global _CACHED_NC
    if _CACHED_NC is None:
        _CACHED_NC = build_bass()
    return _CACHED_NC


def make_in_maps(x, width_positions, height_positions):
    """Shard full inputs into per-core input maps (batch-sharded)."""
    x = np.asarray(x, dtype=np.float32).astype(ml_dtypes.bfloat16)
    wp = np.asarray(width_positions, dtype=np.int32)
    hp = np.asarray(height_positions, dtype=np.int32)
    in_maps = []
    for rr in range(N_CORES):
        sl = slice(rr * BL, (rr + 1) * BL)
        # [BL,C,NB,2] -> [NI, NB] per kind
        ws = wp[sl, :, :, 0].reshape(NI, NB)
        we = wp[sl, :, :, 1].reshape(NI, NB)
        hs = hp[sl, :, :, 0].reshape(NI, NB)
        he = hp[sl, :, :, 1].reshape(NI, NB)
        mw = np.zeros((128, NT, 2 * W), _F8)
        mh = np.zeros((128, NT, H), _F8)
        for p in range(NP):
            T, q = divmod(p, 4)
            a, b = 2 * p, 2 * p + 1
            base = 32 * q
            for k in range(NB):
                mw[base + k, T, ws[a, k] : we[a, k]] = 1
                mw[base + NB + k, T, W + ws[b, k] : W + we[b, k]] = 1
                mh[base + k, T, hs[a, k] : he[a, k]] = 1
                mh[base + NB + k, T, hs[b, k] : he[b, k]] = 1
        in_maps.append({"x": np.ascontiguousarray(x[sl]), "mw": mw, "mh": mh})
    return in_maps


def run(x, width_positions, height_positions, trace=False, tmpdir=None):
    """Run on 8 NeuronCores; returns (full_output, BassKernelResults)."""
    nc = _get_nc()
    in_maps = make_in_maps(x, width_positions, height_positions)
    res = run_bass_kernel_spmd(
        nc, in_maps, core_ids=list(range(N_CORES)), trace=trace, tmpdir=tmpdir
    )
    out = np.concatenate(
        [np.asarray(r["out"]).astype(np.float32) for r in res.results], axis=0
    )
    return out, res


def kernel(x, width_positions, height_positions):
    out, _ = run(x, width_positions, height_positions)
    return out
